# revision 30
# baseline (speedup 1.0000x reference)
"""Trainium2 Bass kernel for DocREModel_KD head (ragged_sequence).

Problem shape (hardcoded, per spec):
  sequence_output [4, 1024, 768] f32
  attention       [4, 12, 1024, 1024] f32
  entity_starts   [4, 42, 4] int
  hts             [4, 1764, 2] int
Outputs: (hss, rss, tss) each [4, 42, 42, 768] f32.

Strategy v7 (8 cores, SPMD single program, c-split + host reduce):
  - 2 cores per document, split by the attention column dim c (512 each).
    Each core gathers only its c-half of the mention attention rows (staged
    host-side as bf16 [pos, (chunk, h, c128)] so the gather lands per
    c-chunk and the pipeline starts after the first ~260KB), computes the
    full canonical pair grid G over its c-half, and emits UNNORMALIZED
    per-chunk partial rs plus a partial normalizer via a ones-column. The
    host sums partials over chunks and cores and normalizes (unshard).
  - Canonical pair packing: 7 i-blocks of height 6, block b covers
    j in [6b, 42): U = 1008 rows, padded to 8 taus of 128 (FWL weights).
  - EA (mention-mean of attention, c-partitioned) via tiny PE matmuls
    against an [84, 21] 0.25-selection matrix (mean + transpose in one
    step); the ACT drain un-interleaves h-major PSUM bands to the h-minor
    layout the DVE pair products need for 2x mode.
  - Pair products on DVE (bf16 2x) + the 12->6 reduction level on DVE;
    the 6->1 reduction runs on the Tensor engine as identity-weight
    accumulating matmuls into PSUM (f32, exact), with relu fused into the
    ACT drain.
  - rs partials per (tau, chunk): single matmuls, no cross-chunk PSUM
    liveness, so rs overlaps the DVE pipeline chunk by chunk; output DMAs
    alternate between the Sync and GpSimd queues.
  - e_emb logsumexp d-split across the core pair (exact fp32).
  - hss/tss and the hts->grid mapping assembled host-side.
"""

import numpy as np
from contextlib import ExitStack

import concourse.bass as bass
import concourse.bacc as bacc
import concourse.mybir as mybir
import concourse.tile as tile
from concourse.bass_utils import run_bass_kernel_spmd

# ---- problem constants ----
B, H, C, HS, NE, M = 4, 12, 1024, 768, 42, 4
OFFSET = 1
CH = C // 2          # 512: c-half per core
NCH = CH // 128      # 4 c-chunks per core
BH = 6               # i-block height
NB = NE // BH        # 7 blocks
BLKW = [NE - BH * b for b in range(NB)]            # 42,36,30,24,18,12,6
BLKOFF = [BH * sum(BLKW[:b]) for b in range(NB)]   # packed row offsets
U = BH * sum(BLKW)   # 1008 packed canonical pair rows
UH = U // 2          # 504: tree/relu half width
UTAU = 128           # padded tau width (G padded to 1024 rows for FWL)
NTAU_P = 8
GT = 84              # mentions per gather tile (21 entities x 4)
WLSE = HS // 2       # 384: e_emb d-split width per core
N_CORES = 8

F32 = mybir.dt.float32
BF16 = mybir.dt.bfloat16
I32 = mybir.dt.int32
NP_BF16 = mybir.dt.np(BF16)

_prog_cache = {}


def _build_program():
    nc = bacc.Bacc(None)

    # att per c-chunk: [pos, (h, c128)] rows of 3KB
    att_ks = [
        nc.dram_tensor(f"att{k}", [C, H * 128], BF16, kind="ExternalInput")
        for k in range(NCH)
    ]
    seq_b = nc.dram_tensor("seq_b", [CH, HS], BF16, kind="ExternalInput")
    seq_lse = nc.dram_tensor("seq_lse", [C, WLSE], F32, kind="ExternalInput")
    sel_d = nc.dram_tensor("sel", [GT, NE // 2], BF16, kind="ExternalInput")
    ident_d = nc.dram_tensor("ident", [128, 128], BF16, kind="ExternalInput")
    idx_g_d = nc.dram_tensor("idx_g", [GT, 2], I32, kind="ExternalInput")
    idx_lse_d = nc.dram_tensor("idx_lse", [NE, M], I32, kind="ExternalInput")

    # per-chunk unnormalized rs partials (+ ones-column), host-reduced
    rs_out = nc.dram_tensor(
        "rs_out", [NCH * NTAU_P * UTAU, HS + 1], BF16, kind="ExternalOutput"
    )
    eemb_out = nc.dram_tensor("eemb_out", [NE, WLSE], F32, kind="ExternalOutput")

    with tile.TileContext(nc) as tc, ExitStack() as ctx:
        const_p = ctx.enter_context(tc.tile_pool(name="const", bufs=1))
        raw_p = ctx.enter_context(tc.tile_pool(name="raw", bufs=1))
        ea_p = ctx.enter_context(tc.tile_pool(name="ea", bufs=1))
        pr_p = ctx.enter_context(tc.tile_pool(name="pr", bufs=2))
        t6_p = ctx.enter_context(tc.tile_pool(name="t6", bufs=2))
        g_p = ctx.enter_context(tc.tile_pool(name="g", bufs=1))
        seqb_p = ctx.enter_context(tc.tile_pool(name="seqb", bufs=1))
        lse_p = ctx.enter_context(tc.tile_pool(name="lse", bufs=1))
        st_p = ctx.enter_context(tc.tile_pool(name="st", bufs=4))

        ea_ps = ctx.enter_context(tc.tile_pool(name="eaps", bufs=2, space="PSUM"))
        tr_ps = ctx.enter_context(tc.tile_pool(name="trps", bufs=2, space="PSUM"))
        rsA_ps = ctx.enter_context(tc.tile_pool(name="rsA", bufs=2, space="PSUM"))
        rsB_ps = ctx.enter_context(tc.tile_pool(name="rsB", bufs=2, space="PSUM"))

        # --- constants / indices to SBUF ---
        ig_sb = const_p.tile([GT, 2], I32, name="ig_sb")
        nc.sync.dma_start(out=ig_sb[:], in_=idx_g_d[:])
        il_sb = const_p.tile([NE, M], I32, name="il_sb")
        nc.sync.dma_start(out=il_sb[:], in_=idx_lse_d[:])
        sel_sb = const_p.tile([GT, NE // 2], BF16, name="sel_sb")
        nc.sync.dma_start(out=sel_sb[:], in_=sel_d[:])
        id_sb = const_p.tile([128, 128], BF16, name="id_sb")
        nc.sync.dma_start(out=id_sb[:], in_=ident_d[:])

        # --- indirect gathers: per (mention-tile, c-chunk), 3KB rows ---
        raws = [[None] * NCH, [None] * NCH]
        for k in range(NCH):
            for t in range(2):
                rt = raw_p.tile([GT, H * 128], BF16, name=f"raw{t}{k}")
                nc.gpsimd.indirect_dma_start(
                    out=rt[:],
                    out_offset=None,
                    in_=att_ks[k][:],
                    in_offset=bass.IndirectOffsetOnAxis(ap=ig_sb[:, t : t + 1], axis=0),
                )
                raws[t][k] = rt

        # --- e_emb logsumexp pipeline (d-split half, exact fp32) ---
        sg = []
        for r in range(M):
            g = lse_p.tile([NE, WLSE], F32, name=f"sg{r}")
            nc.gpsimd.indirect_dma_start(
                out=g[:],
                out_offset=None,
                in_=seq_lse[:],
                in_offset=bass.IndirectOffsetOnAxis(ap=il_sb[:, r : r + 1], axis=0),
            )
            sg.append(g)
        ex = []
        for r in range(M):
            e = lse_p.tile([NE, WLSE], F32, name=f"ex{r}")
            nc.scalar.activation(out=e[:], in_=sg[r][:], func=mybir.ActivationFunctionType.Exp)
            ex.append(e)
        s01 = lse_p.tile([NE, WLSE], F32, name="s01")
        s23 = lse_p.tile([NE, WLSE], F32, name="s23")
        nc.vector.tensor_add(out=s01[:], in0=ex[0][:], in1=ex[1][:])
        nc.vector.tensor_add(out=s23[:], in0=ex[2][:], in1=ex[3][:])
        nc.vector.tensor_add(out=s01[:], in0=s01[:], in1=s23[:])
        lse_res = lse_p.tile([NE, WLSE], F32, name="lse_res")
        nc.scalar.activation(out=lse_res[:], in_=s01[:], func=mybir.ActivationFunctionType.Ln)
        nc.scalar.dma_start(out=eemb_out[:], in_=lse_res[:])

        # --- sequence chunks (already bf16) + ones column ---
        seqb = []
        for k in range(NCH):
            sb = seqb_p.tile([128, HS + 1], BF16, name=f"sb{k}")
            nc.sync.dma_start(out=sb[:, 0:HS], in_=seq_b[k * 128 : (k + 1) * 128, :])
            nc.vector.memset(sb[:, HS : HS + 1], 1.0)
            seqb.append(sb)

        # --- EA for all chunks up front (PE only needs the gathers) ---
        eas = []
        for k in range(NCH):
            ps = ea_ps.tile([128, H * NE], F32, name="eaps")
            for t in range(2):
                for h in range(H):
                    nc.tensor.matmul(
                        out=ps[:, h * NE + t * 21 : h * NE + t * 21 + 21],
                        lhsT=raws[t][k][:, h * 128 : (h + 1) * 128],
                        rhs=sel_sb[:],
                        start=True,
                        stop=True,
                    )
            # drain + un-interleave to h-minor bf16 [128, (n, h)]
            ea = ea_p.tile([128, NE * H], BF16, name=f"ea{k}")
            nc.scalar.activation(
                out=ea[:].rearrange("p (n h) -> p n h", h=H),
                in_=ps[:].rearrange("p (h n) -> p n h", n=NE),
                func=mybir.ActivationFunctionType.Copy,
            )
            eas.append(ea)

        # --- per chunk: products (DVE), 12->6 (DVE), 6->1 (PE), rs ---
        for k in range(NCH):
            ea3 = eas[k][:].rearrange("p (n h) -> p n h", h=H)
            pr = pr_p.tile([128, U * H], BF16, name="pr")
            for b in range(NB):
                w = BLKW[b]
                jf = BH * b
                in0 = ea3[:, jf : jf + BH, :].unsqueeze(2).to_broadcast([128, BH, w, H])
                in1 = ea3[:, jf:NE, :].unsqueeze(1).to_broadcast([128, BH, w, H])
                sec = pr[:, BLKOFF[b] * H : (BLKOFF[b] + BH * w) * H]
                pr4 = sec.rearrange("p (i j h) -> p i j h", j=w, h=H)
                nc.vector.tensor_tensor(out=pr4, in0=in0, in1=in1, op=mybir.AluOpType.mult)

            # L1: 12 -> 6 on DVE (2x, aligned)
            pru = pr[:].rearrange("p (u h) -> p u h", h=H)
            t6 = t6_p.tile([128, U * 6], BF16, name="t6")
            t6v = t6[:].rearrange("p (u s) -> p u s", s=6)
            nc.vector.tensor_tensor(out=t6v, in0=pru[:, :, 0:6], in1=pru[:, :, 6:12], op=mybir.AluOpType.add)

            # 6 -> 1 on PE: identity-weight accumulating matmuls (f32 PSUM),
            # relu fused into the ACT drain; G padded to 1024 for FWL taus
            g_t = g_p.tile([128, NTAU_P * UTAU], BF16, name=f"g{k}")
            if k == 0:
                pass
            nc.gpsimd.memset(g_t[:, U:], 0.0)
            for half in range(2):
                psT = tr_ps.tile([128, UH], F32, name="psT")
                for s in range(6):
                    rhs = t6v[:, half * UH : (half + 1) * UH, s : s + 1].squeeze(2)
                    nc.tensor.matmul(
                        out=psT[:], lhsT=id_sb[:], rhs=rhs,
                        start=(s == 0), stop=(s == 5),
                    )
                nc.scalar.activation(
                    out=g_t[:, half * UH : (half + 1) * UH], in_=psT[:],
                    func=mybir.ActivationFunctionType.Relu,
                )

            # rs partials for this chunk: single matmuls per tau
            for tau in range(NTAU_P):
                lo = tau * UTAU
                psA = rsA_ps.tile([UTAU, 512], F32, name="psA")
                psB = rsB_ps.tile([UTAU, HS + 1 - 512], F32, name="psB")
                nc.tensor.matmul(
                    out=psA[:], lhsT=g_t[:, lo : lo + UTAU],
                    rhs=seqb[k][:, 0:512], start=True, stop=True,
                )
                nc.tensor.matmul(
                    out=psB[:], lhsT=g_t[:, lo : lo + UTAU],
                    rhs=seqb[k][:, 512 : HS + 1], start=True, stop=True,
                )
                st = st_p.tile([UTAU, HS + 1], BF16, name="st")
                nc.scalar.activation(
                    out=st[:, 0:512], in_=psA[:],
                    func=mybir.ActivationFunctionType.Copy,
                )
                nc.scalar.activation(
                    out=st[:, 512 : HS + 1], in_=psB[:],
                    func=mybir.ActivationFunctionType.Copy,
                )
                glob = (k * NTAU_P + tau) * UTAU
                eng = nc.sync if tau % 2 == 0 else nc.gpsimd
                eng.dma_start(out=rs_out[glob : glob + UTAU, :], in_=st[:])

    nc.finalize()
    return nc


def _host_inputs(sequence_output, attention, entity_starts):
    """Build the 8 per-core input maps."""
    sel_np = np.zeros([GT, NE // 2], np.float32)
    sel_np[np.arange(GT), np.arange(GT) // M] = 0.25
    sel_np = sel_np.astype(NP_BF16)
    ident_np = np.eye(128, dtype=np.float32).astype(NP_BF16)

    in_maps = []
    for d in range(B):
        starts_doc = np.asarray(entity_starts[d], dtype=np.int64)
        pos = (starts_doc + OFFSET).astype(np.int32)      # [42, 4], < 1024

        ig = np.zeros([GT, 2], np.int32)
        for t in range(2):
            ig[:, t] = pos[21 * t + np.arange(GT) // M, np.arange(GT) % M]

        att_bf = np.asarray(attention[d], dtype=np.float32).astype(NP_BF16)  # [12,1024,1024]
        att_t = att_bf.transpose(1, 0, 2)                 # [pos, h, c]
        seq_doc = np.asarray(sequence_output[d], dtype=np.float32)

        for ch in range(2):
            csl = slice(ch * CH, (ch + 1) * CH)
            # [pos, h, c512] -> [pos, chunk, h, c128]
            a = np.ascontiguousarray(att_t[:, :, csl]).reshape(C, H, NCH, 128)
            a = np.ascontiguousarray(a.transpose(0, 2, 1, 3))  # [pos, k, h, 128]
            im = {
                "seq_b": np.ascontiguousarray(seq_doc[csl, :]).astype(NP_BF16),
                "seq_lse": np.ascontiguousarray(
                    seq_doc[:, ch * WLSE : (ch + 1) * WLSE]
                ),
                "sel": sel_np,
                "ident": ident_np,
                "idx_g": ig,
                "idx_lse": pos,
            }
            for k in range(NCH):
                im[f"att{k}"] = np.ascontiguousarray(a[:, k]).reshape(C, H * 128)
            in_maps.append(im)
    return in_maps


_row_table_cache = {}


def _grid_row_table():
    """[42, 42] -> packed canonical row (use at [min, max])."""
    if "t" not in _row_table_cache:
        row_of = np.full((NE, NE), -1, np.int64)
        for bb in range(NB):
            w = BLKW[bb]
            jf = BH * bb
            for il in range(BH):
                for j in range(jf, NE):
                    row_of[jf + il, j] = BLKOFF[bb] + il * w + (j - jf)
        _row_table_cache["t"] = row_of
    return _row_table_cache["t"]


def _assemble(results, hts):
    eemb = np.empty([B, NE, HS], np.float32)
    rs_rows = np.empty([B, U, HS], np.float32)
    row_of = _grid_row_table()
    for d in range(B):
        o0 = results[2 * d]["eemb_out"]
        o1 = results[2 * d + 1]["eemb_out"]
        eemb[d, :, 0:WLSE] = o0
        eemb[d, :, WLSE:HS] = o1

        p0 = np.asarray(results[2 * d]["rs_out"], dtype=np.float32)
        p1 = np.asarray(results[2 * d + 1]["rs_out"], dtype=np.float32)
        s = (p0 + p1).reshape(NCH, NTAU_P * UTAU, HS + 1).sum(axis=0)[:U]
        rs_rows[d] = s[:, 0:HS] / (s[:, HS : HS + 1] + 1e-10)

    hts_np = np.asarray(hts, dtype=np.int64)
    h_idx = hts_np[:, :, 0]                            # [B, 1764]
    t_idx = hts_np[:, :, 1]
    mn = np.minimum(h_idx, t_idx)
    mx = np.maximum(h_idx, t_idx)
    shape = (B, NE, NE, HS)
    hss = np.empty([B, NE * NE, HS], np.float32)
    rss = np.empty([B, NE * NE, HS], np.float32)
    tss = np.empty([B, NE * NE, HS], np.float32)
    for d in range(B):
        hss[d] = eemb[d][h_idx[d]]
        tss[d] = eemb[d][t_idx[d]]
        rss[d] = rs_rows[d][row_of[mn[d], mx[d]]]
    return hss.reshape(shape), rss.reshape(shape), tss.reshape(shape)


def kernel(sequence_output, attention, entity_starts, hts):
    if "nc" not in _prog_cache:
        _prog_cache["nc"] = _build_program()
    nc = _prog_cache["nc"]

    in_maps = _host_inputs(sequence_output, attention, entity_starts)
    res = run_bass_kernel_spmd(nc, in_maps, list(range(N_CORES))).results
    return _assemble(res, hts)


if __name__ == "__main__":
    # smoke test with random data
    rng = np.random.default_rng(0)
    seq = rng.standard_normal((B, C, HS), dtype=np.float32)
    att = rng.random((B, H, C, C), dtype=np.float32)
    starts = rng.integers(0, 1020, (B, NE, M))
    hts_a = rng.integers(0, NE, (B, NE * NE, 2))
    outs = kernel(seq, att, starts, hts_a)
    print([o.shape for o in outs])


# revision 37
# speedup vs baseline: 1.2410x; 1.2410x over previous
"""Trainium2 Bass kernel for DocREModel_KD head (ragged_sequence).

Problem shape (hardcoded, per spec):
  sequence_output [4, 1024, 768] f32
  attention       [4, 12, 1024, 1024] f32
  entity_starts   [4, 42, 4] int
  hts             [4, 1764, 2] int
Outputs: (hss, rss, tss) each [4, 42, 42, 768] f32.

Strategy v7 (8 cores, SPMD single program, c-split + host reduce):
  - 2 cores per document, split by the attention column dim c (512 each).
    Each core gathers only its c-half of the mention attention rows (staged
    host-side as bf16 [pos, (chunk, h, c128)] so the gather lands per
    c-chunk and the pipeline starts after the first ~260KB), computes the
    full canonical pair grid G over its c-half, and emits UNNORMALIZED
    per-chunk partial rs plus a partial normalizer via a ones-column. The
    host sums partials over chunks and cores and normalizes (unshard).
  - Canonical pair packing: 7 i-blocks of height 6, block b covers
    j in [6b, 42): U = 1008 rows, padded to 8 taus of 128 (FWL weights).
  - EA (mention-mean of attention, c-partitioned) via tiny PE matmuls
    against an [84, 21] 0.25-selection matrix (mean + transpose in one
    step); the ACT drain un-interleaves h-major PSUM bands to the h-minor
    layout the DVE pair products need for 2x mode.
  - Pair products on DVE (bf16 2x) + the 12->6 reduction level on DVE;
    the 6->1 reduction runs on the Tensor engine as identity-weight
    accumulating matmuls into PSUM (f32, exact), with relu fused into the
    ACT drain.
  - rs partials per (tau, chunk): single matmuls, no cross-chunk PSUM
    liveness, so rs overlaps the DVE pipeline chunk by chunk; output DMAs
    alternate between the Sync and GpSimd queues.
  - e_emb logsumexp d-split across the core pair (exact fp32).
  - hss/tss and the hts->grid mapping assembled host-side.
"""

import numpy as np
from contextlib import ExitStack

import concourse.bass as bass
import concourse.bacc as bacc
import concourse.mybir as mybir
import concourse.tile as tile
from concourse.bass_utils import run_bass_kernel_spmd

# ---- problem constants ----
B, H, C, HS, NE, M = 4, 12, 1024, 768, 42, 4
OFFSET = 1
CH = C // 2          # 512: c-half per core
NCH = CH // 128      # 4 c-chunks per core
BH = 6               # i-block height
NB = NE // BH        # 7 blocks
BLKW = [NE - BH * b for b in range(NB)]            # 42,36,30,24,18,12,6
BLKOFF = [BH * sum(BLKW[:b]) for b in range(NB)]   # packed row offsets
U = BH * sum(BLKW)   # 1008 packed canonical pair rows
UH = U // 2          # 504: tree/relu half width
UTAU = 128           # padded tau width (G padded to 1024 rows for FWL)
NTAU_P = 8
GT = 84              # mentions per gather tile (21 entities x 4)
WLSE = HS // 2       # 384: e_emb d-split width per core
N_CORES = 8

F32 = mybir.dt.float32
BF16 = mybir.dt.bfloat16
I32 = mybir.dt.int32
NP_BF16 = mybir.dt.np(BF16)

_prog_cache = {}


def _build_program():
    nc = bacc.Bacc(None)

    # att per c-chunk: [pos, (h, c128)] rows of 3KB
    att_ks = [
        nc.dram_tensor(f"att{k}", [C, H * 128], BF16, kind="ExternalInput")
        for k in range(NCH)
    ]
    seq_b = nc.dram_tensor("seq_b", [CH, HS], BF16, kind="ExternalInput")
    seq_lse = nc.dram_tensor("seq_lse", [C, WLSE], F32, kind="ExternalInput")
    sel_d = nc.dram_tensor("sel", [GT, NE // 2], BF16, kind="ExternalInput")
    idx_g_d = nc.dram_tensor("idx_g", [GT, 2], I32, kind="ExternalInput")
    idx_lse_d = nc.dram_tensor("idx_lse", [NE, M], I32, kind="ExternalInput")

    # per-chunk-pair unnormalized rs partials (+ ones-column), host-reduced
    rs_out = nc.dram_tensor(
        "rs_out", [(NCH // 2) * NTAU_P * UTAU, HS + 1], BF16, kind="ExternalOutput"
    )
    eemb_out = nc.dram_tensor("eemb_out", [NE, WLSE], F32, kind="ExternalOutput")

    with tile.TileContext(nc) as tc, ExitStack() as ctx:
        const_p = ctx.enter_context(tc.tile_pool(name="const", bufs=1))
        raw_p = ctx.enter_context(tc.tile_pool(name="raw", bufs=1))
        ea_p = ctx.enter_context(tc.tile_pool(name="ea", bufs=1))
        pr_p = ctx.enter_context(tc.tile_pool(name="pr", bufs=2))
        t6_p = ctx.enter_context(tc.tile_pool(name="t6", bufs=2))
        x2_p = ctx.enter_context(tc.tile_pool(name="x2", bufs=2))
        gs_p = ctx.enter_context(tc.tile_pool(name="gs", bufs=2))
        g_p = ctx.enter_context(tc.tile_pool(name="g", bufs=1))
        seqb_p = ctx.enter_context(tc.tile_pool(name="seqb", bufs=1))
        lse_p = ctx.enter_context(tc.tile_pool(name="lse", bufs=1))
        st_p = ctx.enter_context(tc.tile_pool(name="st", bufs=3))

        ea_ps = ctx.enter_context(tc.tile_pool(name="eaps", bufs=2, space="PSUM"))
        rsA_ps = ctx.enter_context(tc.tile_pool(name="rsA", bufs=3, space="PSUM"))
        rsB_ps = ctx.enter_context(tc.tile_pool(name="rsB", bufs=3, space="PSUM"))

        # --- constants / indices to SBUF ---
        ig_sb = const_p.tile([GT, 2], I32, name="ig_sb")
        nc.sync.dma_start(out=ig_sb[:], in_=idx_g_d[:])
        il_sb = const_p.tile([NE, M], I32, name="il_sb")
        nc.sync.dma_start(out=il_sb[:], in_=idx_lse_d[:])
        sel_sb = const_p.tile([GT, NE // 2], BF16, name="sel_sb")
        nc.sync.dma_start(out=sel_sb[:], in_=sel_d[:])

        # --- indirect gathers: per (mention-tile, c-chunk), 3KB rows ---
        raws = [[None] * NCH, [None] * NCH]
        for k in range(NCH):
            for t in range(2):
                rt = raw_p.tile([GT, H * 128], BF16, name=f"raw{t}{k}")
                nc.gpsimd.indirect_dma_start(
                    out=rt[:],
                    out_offset=None,
                    in_=att_ks[k][:],
                    in_offset=bass.IndirectOffsetOnAxis(ap=ig_sb[:, t : t + 1], axis=0),
                )
                raws[t][k] = rt

        # --- e_emb logsumexp pipeline (d-split half, exact fp32) ---
        sg = []
        for r in range(M):
            g = lse_p.tile([NE, WLSE], F32, name=f"sg{r}")
            nc.gpsimd.indirect_dma_start(
                out=g[:],
                out_offset=None,
                in_=seq_lse[:],
                in_offset=bass.IndirectOffsetOnAxis(ap=il_sb[:, r : r + 1], axis=0),
            )
            sg.append(g)
        ex = []
        for r in range(M):
            e = lse_p.tile([NE, WLSE], F32, name=f"ex{r}")
            nc.scalar.activation(out=e[:], in_=sg[r][:], func=mybir.ActivationFunctionType.Exp)
            ex.append(e)
        s01 = lse_p.tile([NE, WLSE], F32, name="s01")
        s23 = lse_p.tile([NE, WLSE], F32, name="s23")
        nc.vector.tensor_add(out=s01[:], in0=ex[0][:], in1=ex[1][:])
        nc.vector.tensor_add(out=s23[:], in0=ex[2][:], in1=ex[3][:])
        nc.vector.tensor_add(out=s01[:], in0=s01[:], in1=s23[:])
        lse_res = lse_p.tile([NE, WLSE], F32, name="lse_res")
        nc.scalar.activation(out=lse_res[:], in_=s01[:], func=mybir.ActivationFunctionType.Ln)
        nc.scalar.dma_start(out=eemb_out[:], in_=lse_res[:])

        # --- sequence chunks (already bf16) + ones column ---
        seqb = []
        for k in range(NCH):
            sb = seqb_p.tile([128, HS + 1], BF16, name=f"sb{k}")
            nc.sync.dma_start(out=sb[:, 0:HS], in_=seq_b[k * 128 : (k + 1) * 128, :])
            nc.vector.memset(sb[:, HS : HS + 1], 1.0)
            seqb.append(sb)

        # --- EA for all chunks up front (PE only needs the gathers) ---
        eas = []
        for k in range(NCH):
            ps = ea_ps.tile([128, H * NE], F32, name="eaps")
            for t in range(2):
                for h in range(H):
                    nc.tensor.matmul(
                        out=ps[:, h * NE + t * 21 : h * NE + t * 21 + 21],
                        lhsT=raws[t][k][:, h * 128 : (h + 1) * 128],
                        rhs=sel_sb[:],
                        start=True,
                        stop=True,
                    )
            # drain + un-interleave to h-minor bf16 [128, (n, h)]
            ea = ea_p.tile([128, NE * H], BF16, name=f"ea{k}")
            nc.scalar.activation(
                out=ea[:].rearrange("p (n h) -> p n h", h=H),
                in_=ps[:].rearrange("p (h n) -> p n h", n=NE),
                func=mybir.ActivationFunctionType.Copy,
            )
            eas.append(ea)

        # --- per chunk: products (DVE), reduction tree (DVE), relu (ACT) ---
        gs = []
        for k in range(NCH):
            ea3 = eas[k][:].rearrange("p (n h) -> p n h", h=H)
            pr = pr_p.tile([128, U * H], BF16, name="pr")
            for b in range(NB):
                w = BLKW[b]
                jf = BH * b
                in0 = ea3[:, jf : jf + BH, :].unsqueeze(2).to_broadcast([128, BH, w, H])
                in1 = ea3[:, jf:NE, :].unsqueeze(1).to_broadcast([128, BH, w, H])
                sec = pr[:, BLKOFF[b] * H : (BLKOFF[b] + BH * w) * H]
                pr4 = sec.rearrange("p (i j h) -> p i j h", j=w, h=H)
                nc.vector.tensor_tensor(out=pr4, in0=in0, in1=in1, op=mybir.AluOpType.mult)

            # L1: 12 -> 6 ; L2: 6 -> 2 (4B-aligned pairs) ; L3: 2 -> 1
            pru = pr[:].rearrange("p (u h) -> p u h", h=H)
            t6 = t6_p.tile([128, U * 6], BF16, name="t6")
            t6v = t6[:].rearrange("p (u s) -> p u s", s=6)
            nc.vector.tensor_tensor(out=t6v, in0=pru[:, :, 0:6], in1=pru[:, :, 6:12], op=mybir.AluOpType.add)
            x2 = x2_p.tile([128, U * 2], BF16, name="x2")
            x2v = x2[:].rearrange("p (u s) -> p u s", s=2)
            nc.vector.tensor_tensor(out=x2v, in0=t6v[:, :, 0:2], in1=t6v[:, :, 2:4], op=mybir.AluOpType.add)
            nc.vector.tensor_tensor(out=x2v, in0=x2v, in1=t6v[:, :, 4:6], op=mybir.AluOpType.add)
            gsum = gs_p.tile([128, U], BF16, name="gsum")
            nc.vector.tensor_tensor(
                out=gsum[:], in0=x2v[:, :, 0:1].squeeze(2),
                in1=x2v[:, :, 1:2].squeeze(2), op=mybir.AluOpType.add,
            )

            # relu on ACT; pad to 1024 rows so rs taus are 128 wide (FWL)
            g_t = g_p.tile([128, NTAU_P * UTAU], BF16, name=f"g{k}")
            nc.gpsimd.memset(g_t[:, U:], 0.0)
            nc.scalar.activation(out=g_t[:, 0:U], in_=gsum[:], func=mybir.ActivationFunctionType.Relu)
            gs.append(g_t)

        # rs partials accumulated over chunk pairs; 2 taus share one output
        # tile so the result DMAs are few and large
        for kp in range(NCH // 2):
            for tp in range(NTAU_P // 2):
                psAs, psBs = {}, {}
                sts = st_p.tile([UTAU, 2 * (HS + 1)], BF16, name="st")
                for j in range(2):
                    tau = 2 * tp + j
                    lo = tau * UTAU
                    psA = rsA_ps.tile([UTAU, 512], F32, name="psA")
                    psB = rsB_ps.tile([UTAU, HS + 1 - 512], F32, name="psB")
                    for k in (2 * kp, 2 * kp + 1):
                        nc.tensor.matmul(
                            out=psA[:], lhsT=gs[k][:, lo : lo + UTAU],
                            rhs=seqb[k][:, 0:512],
                            start=(k == 2 * kp), stop=(k == 2 * kp + 1),
                        )
                        nc.tensor.matmul(
                            out=psB[:], lhsT=gs[k][:, lo : lo + UTAU],
                            rhs=seqb[k][:, 512 : HS + 1],
                            start=(k == 2 * kp), stop=(k == 2 * kp + 1),
                        )
                    off = j * (HS + 1)
                    nc.scalar.activation(
                        out=sts[:, off : off + 512], in_=psA[:],
                        func=mybir.ActivationFunctionType.Copy,
                    )
                    nc.scalar.activation(
                        out=sts[:, off + 512 : off + HS + 1], in_=psB[:],
                        func=mybir.ActivationFunctionType.Copy,
                    )
                glob = (kp * NTAU_P + 2 * tp) * UTAU
                eng = nc.sync if tp % 2 == 0 else nc.gpsimd
                eng.dma_start(
                    out=rs_out[glob : glob + 2 * UTAU, :].rearrange(
                        "(j p) f -> p j f", j=2
                    ),
                    in_=sts[:].rearrange("p (j f) -> p j f", j=2),
                )

    nc.finalize()
    return nc


def _host_inputs(sequence_output, attention, entity_starts):
    """Build the 8 per-core input maps."""
    sel_np = np.zeros([GT, NE // 2], np.float32)
    sel_np[np.arange(GT), np.arange(GT) // M] = 0.25
    sel_np = sel_np.astype(NP_BF16)

    in_maps = []
    for d in range(B):
        starts_doc = np.asarray(entity_starts[d], dtype=np.int64)
        pos = (starts_doc + OFFSET).astype(np.int32)      # [42, 4], < 1024

        ig = np.zeros([GT, 2], np.int32)
        for t in range(2):
            ig[:, t] = pos[21 * t + np.arange(GT) // M, np.arange(GT) % M]

        att_bf = np.asarray(attention[d], dtype=np.float32).astype(NP_BF16)  # [12,1024,1024]
        att_t = att_bf.transpose(1, 0, 2)                 # [pos, h, c]
        seq_doc = np.asarray(sequence_output[d], dtype=np.float32)

        for ch in range(2):
            csl = slice(ch * CH, (ch + 1) * CH)
            # [pos, h, c512] -> [pos, chunk, h, c128]
            a = np.ascontiguousarray(att_t[:, :, csl]).reshape(C, H, NCH, 128)
            a = np.ascontiguousarray(a.transpose(0, 2, 1, 3))  # [pos, k, h, 128]
            im = {
                "seq_b": np.ascontiguousarray(seq_doc[csl, :]).astype(NP_BF16),
                "seq_lse": np.ascontiguousarray(
                    seq_doc[:, ch * WLSE : (ch + 1) * WLSE]
                ),
                "sel": sel_np,
                "idx_g": ig,
                "idx_lse": pos,
            }
            for k in range(NCH):
                im[f"att{k}"] = np.ascontiguousarray(a[:, k]).reshape(C, H * 128)
            in_maps.append(im)
    return in_maps


_row_table_cache = {}


def _grid_row_table():
    """[42, 42] -> packed canonical row (use at [min, max])."""
    if "t" not in _row_table_cache:
        row_of = np.full((NE, NE), -1, np.int64)
        for bb in range(NB):
            w = BLKW[bb]
            jf = BH * bb
            for il in range(BH):
                for j in range(jf, NE):
                    row_of[jf + il, j] = BLKOFF[bb] + il * w + (j - jf)
        _row_table_cache["t"] = row_of
    return _row_table_cache["t"]


def _assemble(results, hts):
    eemb = np.empty([B, NE, HS], np.float32)
    rs_rows = np.empty([B, U, HS], np.float32)
    row_of = _grid_row_table()
    for d in range(B):
        o0 = results[2 * d]["eemb_out"]
        o1 = results[2 * d + 1]["eemb_out"]
        eemb[d, :, 0:WLSE] = o0
        eemb[d, :, WLSE:HS] = o1

        p0 = np.asarray(results[2 * d]["rs_out"], dtype=np.float32)
        p1 = np.asarray(results[2 * d + 1]["rs_out"], dtype=np.float32)
        s = (p0 + p1).reshape(NCH // 2, NTAU_P * UTAU, HS + 1).sum(axis=0)[:U]
        rs_rows[d] = s[:, 0:HS] / (s[:, HS : HS + 1] + 1e-10)

    hts_np = np.asarray(hts, dtype=np.int64)
    h_idx = hts_np[:, :, 0]                            # [B, 1764]
    t_idx = hts_np[:, :, 1]
    mn = np.minimum(h_idx, t_idx)
    mx = np.maximum(h_idx, t_idx)
    shape = (B, NE, NE, HS)
    hss = np.empty([B, NE * NE, HS], np.float32)
    rss = np.empty([B, NE * NE, HS], np.float32)
    tss = np.empty([B, NE * NE, HS], np.float32)
    for d in range(B):
        hss[d] = eemb[d][h_idx[d]]
        tss[d] = eemb[d][t_idx[d]]
        rss[d] = rs_rows[d][row_of[mn[d], mx[d]]]
    return hss.reshape(shape), rss.reshape(shape), tss.reshape(shape)


def kernel(sequence_output, attention, entity_starts, hts):
    if "nc" not in _prog_cache:
        _prog_cache["nc"] = _build_program()
    nc = _prog_cache["nc"]

    in_maps = _host_inputs(sequence_output, attention, entity_starts)
    res = run_bass_kernel_spmd(nc, in_maps, list(range(N_CORES))).results
    return _assemble(res, hts)


if __name__ == "__main__":
    # smoke test with random data
    rng = np.random.default_rng(0)
    seq = rng.standard_normal((B, C, HS), dtype=np.float32)
    att = rng.random((B, H, C, C), dtype=np.float32)
    starts = rng.integers(0, 1020, (B, NE, M))
    hts_a = rng.integers(0, NE, (B, NE * NE, 2))
    outs = kernel(seq, att, starts, hts_a)
    print([o.shape for o in outs])
